# revision 40
# baseline (speedup 1.0000x reference)
"""Causal self-attention (B=4, S=2048, E=1024, H=16) on 8 TRN2 NeuronCores.

Sharding: data parallel on batch (4) x tensor parallel on heads (2 groups of 8).
Core c handles batch c//2, heads (c%2)*8..(c%2)*8+8. Each core computes its
heads' QKV projections, causal attention, and a partial output projection
(w_proj rows for its heads, with b_proj/2 folded in); core pairs then
ReduceScatter-add the partials so each core finishes half the rows of its
batch. No transposes on device: scores are computed as S^T = K @ Q^T, the
softmax denominator comes from a ones-column appended to V, and O^T stacked
over heads is exactly the lhsT the projection matmul needs.

QKV + attention run in bf16 (the PE's full-rate dtype for the K=64/M=65
attention shapes); the output projection accumulates y^T in float32r.
Work is interleaved: QKV per 512-column slice, then per 512-row q-slice
attention -> projection -> pairwise ReduceScatter, so the collectives and
the PE/ACT pipelines overlap.
"""

import sys

sys.path.insert(0, "/opt/trn_rl_repo")

import ml_dtypes
import numpy as np

import concourse.mybir as mybir
import concourse.tile as tile
from concourse import bacc
from concourse.bass_utils import run_bass_kernel_spmd

B, S, E, H, D = 4, 2048, 1024, 16, 64
P = 128
HPC = 8  # heads per core
NPAIR = HPC // 2  # head pairs per core (2 heads share a 128-partition tile)
HE = HPC * D  # 512: embedding slice owned by one core
NKT = S // P  # 16 k-row tiles
NKS = S // 512  # 4 sequence slices of 512
NEC = E // P  # 8 contraction chunks
DV = D + 1  # V columns per head incl. ones column
f32 = mybir.dt.float32
f32r = mybir.dt.float32r
bf16 = mybir.dt.bfloat16
EXPF = mybir.ActivationFunctionType.Exp

_CACHED = {}


def _build_program():
    nc = bacc.Bacc(None, target_bir_lowering=False)

    xT_d = nc.dram_tensor("xT", [E, S], bf16, kind="ExternalInput")
    wq_d = nc.dram_tensor("wq", [E, HE], bf16, kind="ExternalInput")
    wk_d = nc.dram_tensor("wk", [E, HE], bf16, kind="ExternalInput")
    wv_d = nc.dram_tensor("wv", [E, HE], bf16, kind="ExternalInput")
    wp_d = nc.dram_tensor("wp", [HE, E], f32r, kind="ExternalInput")
    bq_d = nc.dram_tensor("bq", [HE], f32, kind="ExternalInput")
    bk_d = nc.dram_tensor("bk", [HE], f32, kind="ExternalInput")
    bv_d = nc.dram_tensor("bv", [1, HE], f32r, kind="ExternalInput")
    bp_d = nc.dram_tensor("bp", [1, E], f32r, kind="ExternalInput")
    y_d = nc.dram_tensor("y_out", [S // 2, E], f32, kind="ExternalOutput")

    with tile.TileContext(nc) as tc:
        with (
            tc.tile_pool(name="const", bufs=1) as cst,
            tc.tile_pool(name="kt", bufs=1) as ktp,
            tc.tile_pool(name="qt", bufs=1) as qtp,
            tc.tile_pool(name="vx", bufs=1) as vxp,
            tc.tile_pool(name="yt", bufs=1) as ytp,
            tc.tile_pool(name="xs", bufs=2) as xsp,
            tc.tile_pool(name="wgt", bufs=1) as wgp,
            tc.tile_pool(name="wp", bufs=1) as wpp,
            tc.tile_pool(name="pt", bufs=5) as ptp,
            tc.tile_pool(name="sm", bufs=3) as smp,
            tc.tile_pool(name="os", bufs=6) as osp,
            tc.tile_pool(name="yp", bufs=3) as ypp,
            tc.tile_pool(name="ps_a", bufs=2, space="PSUM") as psa,
            tc.tile_pool(name="ps_s", bufs=2, space="PSUM") as pss,
            tc.tile_pool(name="ps_o", bufs=2, space="PSUM") as pso,
            tc.tile_pool(name="dram", bufs=1, space="DRAM") as dramp,
        ):
            # ---- constants -------------------------------------------------
            ones32 = cst.tile([P, P], f32)
            nc.gpsimd.memset(ones32[:], 1.0)
            ones_row = cst.tile([1, P], f32r)
            nc.vector.tensor_copy(ones_row[:], ones32[0:1, :])
            ones65 = cst.tile([65, 64], f32r)
            nc.vector.tensor_copy(ones65[:], ones32[0:65, 0:64])
            onescol = cst.tile([P, HPC], bf16)
            nc.vector.tensor_copy(onescol[:], ones32[:, :HPC])

            # diag mask [128,128]: keep (1.0) where col >= row
            dmask32 = cst.tile([P, P], f32)
            nc.gpsimd.memset(dmask32[:], 1.0)
            nc.gpsimd.affine_select(
                out=dmask32[:],
                in_=dmask32[:],
                compare_op=mybir.AluOpType.is_ge,
                fill=0.0,
                base=0,
                pattern=[[1, P]],
                channel_multiplier=-1,
            )
            dmask = cst.tile([P, P], bf16)
            nc.vector.tensor_copy(dmask[:], dmask32[:])
            # mask3 [128,256]: keep where col - 128 >= row (zeros | diag)
            m3_32 = cst.tile([P, 256], f32)
            nc.gpsimd.memset(m3_32[:], 1.0)
            nc.gpsimd.affine_select(
                out=m3_32[:],
                in_=m3_32[:],
                compare_op=mybir.AluOpType.is_ge,
                fill=0.0,
                base=-128,
                pattern=[[1, 256]],
                channel_multiplier=-1,
            )
            mask3 = cst.tile([P, 256], bf16)
            nc.vector.tensor_copy(mask3[:], m3_32[:])

            # persistent activations
            kT = [ktp.tile([P, S], bf16, tag=f"kt{p}", name=f"kt{p}") for p in range(NPAIR)]
            qT = [qtp.tile([P, S], bf16, tag=f"qt{p}", name=f"qt{p}") for p in range(NPAIR)]
            vx = [vxp.tile([P, HPC * DV], bf16, tag=f"vx{k}", name=f"vx{k}") for k in range(NKT)]
            yT = [ytp.tile([P, S], f32r, tag=f"yt{p}", name=f"yt{p}") for p in range(NPAIR)]

            bq_sb = cst.tile([P, NPAIR], f32)
            nc.sync.dma_start(bq_sb[:], bq_d.rearrange("(o p) -> p o", p=P))
            bk_sb = cst.tile([P, NPAIR], f32)
            nc.sync.dma_start(bk_sb[:], bk_d.rearrange("(o p) -> p o", p=P))
            bv_row = cst.tile([1, HE], f32r)
            nc.sync.dma_start(bv_row[:], bv_d[:])
            bp_row = cst.tile([1, E], f32r)
            nc.sync.dma_start(bp_row[:], bp_d[:])

            # ---- bias broadcast rows -> [128, *] tiles via K=1 matmul ------
            bv_bc = cst.tile([P, HE], f32)
            bp_bc = cst.tile([P, E], f32)
            bvp = psa.tile([P, HE], f32, tag="a")
            nc.tensor.matmul(bvp[:], ones_row[:], bv_row[:], start=True, stop=True)
            nc.vector.tensor_copy(bv_bc[:], bvp[:])
            for i in range(2):
                bpp = psa.tile([P, 512], f32, tag="a")
                nc.tensor.matmul(
                    bpp[:],
                    ones_row[:],
                    bp_row[:, 512 * i : 512 * (i + 1)],
                    start=True,
                    stop=True,
                )
                nc.vector.tensor_copy(bp_bc[:, 512 * i : 512 * (i + 1)], bpp[:])


            # ReduceScatter buffers: 3 full 512-row slices + the last
            # slice split 256/128/128 to shrink the serial tail
            _sizes = [512, 512, 512, 512]
            y_parts = [
                dramp.tile([n, E], f32, name=f"ypart{j}")
                for j, n in enumerate(_sizes)
            ]
            y_halves = [
                dramp.tile([n // 2, E], f32, name=f"yhalf{j}")
                for j, n in enumerate(_sizes)
            ]

            # qkv weights (bf16, resident); xs slice 0 prefetched between
            # wk and wq so the first K matmuls start as early as possible
            wk_sb = wgp.tile([P, NEC, HE], bf16, tag="wk", name="wk_sb")
            wk_r = wk_d.rearrange("(o p) m -> p o m", p=P)
            nc.sync.dma_start(wk_sb[:, 0:2], wk_r[:, 0:2])
            xs0 = xsp.tile([P, NEC, 512], bf16, tag="xs", name="xs0")
            xs0_r = xT_d[:, 0:512].rearrange("(o p) s -> p o s", p=P)
            nc.sync.dma_start(xs0[:, 0:2], xs0_r[:, 0:2])
            nc.sync.dma_start(wk_sb[:, 2:NEC], wk_r[:, 2:NEC])
            nc.sync.dma_start(xs0[:, 2:NEC], xs0_r[:, 2:NEC])
            wq_sb = wgp.tile([P, NEC, HE], bf16, tag="wq", name="wq_sb")
            nc.sync.dma_start(wq_sb[:], wq_d.rearrange("(o p) m -> p o m", p=P))
            wv_sb = wgp.tile([P, NEC, HE], bf16, tag="wv", name="wv_sb")
            nc.sync.dma_start(wv_sb[:], wv_d.rearrange("(o p) m -> p o m", p=P))

            # ---- emission-order software pipeline --------------------------
            # Tile executes ~in emission order per engine, so QKV slice ks+1
            # is emitted in unit-sized chunks between attention group-pairs
            # of q-slice tau=ks: the dense QKV matmuls fill the PE while the
            # ACT-bound attention pipeline runs, keeping the HAM clock warm.
            xs_tiles = {0: xs0}

            def xs_for(ks):
                if ks not in xs_tiles:
                    xs = xsp.tile([P, NEC, 512], bf16, tag="xs", name=f"xs{ks}")
                    nc.sync.dma_start(
                        xs[:],
                        xT_d[:, 512 * ks : 512 * (ks + 1)].rearrange(
                            "(o p) s -> p o s", p=P
                        ),
                    )
                    xs_tiles[ks] = xs
                return xs_tiles[ks]

            def kq_unit(ks, p, w_sb, b_sb, dst):
                xs = xs_for(ks)
                acc = psa.tile([P, 512], f32, tag="a", name="acc")
                for o in range(NEC):
                    nc.tensor.matmul(
                        acc[:],
                        w_sb[:, o, P * p : P * (p + 1)],
                        xs[:, o, :],
                        start=(o == 0),
                        stop=(o == NEC - 1),
                    )
                nc.vector.tensor_tensor(
                    dst[p][:, 512 * ks : 512 * (ks + 1)],
                    acc[:],
                    b_sb[:, p : p + 1].to_broadcast((P, 512)),
                    mybir.AluOpType.add,
                )

            def v_unit(ks, kl):
                xs = xs_for(ks)
                kt_i = 4 * ks + kl
                acc = psa.tile([P, HE], f32, tag="a", name="acc")
                for o in range(NEC):
                    nc.tensor.matmul(
                        acc[:],
                        xs[:, o, P * kl : P * (kl + 1)],
                        wv_sb[:, o, :],
                        start=(o == 0),
                        stop=(o == NEC - 1),
                    )
                vt = vx[kt_i].rearrange("p (h d) -> p h d", d=DV)
                nc.vector.tensor_tensor(
                    vt[:, :, 0:D],
                    acc[:].rearrange("p (h d) -> p h d", d=D),
                    bv_bc[:].rearrange("p (h d) -> p h d", d=D),
                    mybir.AluOpType.add,
                )
                nc.vector.tensor_copy(vt[:, :, D], onescol[:])

            def kq_units(ks):
                units = []
                for p in range(NPAIR):
                    units.append(lambda p=p: kq_unit(ks, p, wk_sb, bk_sb, kT))
                for p in range(NPAIR):
                    units.append(lambda p=p: kq_unit(ks, p, wq_sb, bq_sb, qT))
                return units

            def v_units(ks):
                return [lambda kl=kl: v_unit(ks, kl) for kl in range(4)]

            # staircase: (start col, mask tile, mask col) per sub-diag t
            stair = (
                (0, "d", 0),
                (128, "d", 128),
                (256, "d", 256),
                (256, "m3", 256),
            )
            # normalize in batches of 2: denominators gathered at partitions
            # 0 and 64 (quadrant-aligned for the broadcast matmul rhs), one
            # reciprocal per batch
            pending = []

            def flush_normalize():
                batch = pending[:2]
                del pending[:2]
                dn = smp.tile([65, 512], f32r, tag="dn", name="dn")
                for i, (o_sb, dst) in enumerate(batch):
                    nc.vector.tensor_copy(dn[64 * i : 64 * i + 1, :], o_sb[64:65, :])
                n = 64 * (len(batch) - 1) + 1
                with nc.allow_low_precision(reason="softmax recip"):
                    # one call over all gathered rows; the gap rows hold
                    # garbage and are reciprocal'd harmlessly
                    nc.vector.reciprocal(dn[0:n, :], dn[0:n, :])
                for i, (o_sb, dst) in enumerate(batch):
                    bc_ps = psa.tile([P, 512], f32, tag="a", name="bc_ps")
                    nc.tensor.matmul(
                        bc_ps[0:64, :],
                        ones65[64 * i : 64 * i + 1, :],
                        dn[64 * i : 64 * i + 1, :],
                        start=True,
                        stop=True,
                    )
                    bc_sb = smp.tile([64, 512], f32, tag="bcs")
                    nc.vector.tensor_copy(bc_sb[:], bc_ps[0:64, :])
                    nc.vector.tensor_mul(dst, o_sb[0:64, :], bc_sb[:])

            def s_pair(p, h, tau, a, o_ps):
                # S^T for chunks (a, a+1) -> one 2-bank psum tile, one Exp
                qs = 512 * tau
                q_ap = qT[p][64 * h : 64 * h + 64, qs : qs + 512]
                t0, t1 = a - 4 * tau, a + 1 - 4 * tau
                cols = (
                    0 if t0 < 0 else stair[t0][0],
                    0 if t1 < 0 else stair[t1][0],
                )
                s2 = pss.tile([P, 1024], f32, tag="s", name="s2")
                p2 = ptp.tile([P, 1024], bf16, tag="p", name="p2")
                for idx in range(2):
                    j = a + idx
                    off = 512 * idx
                    c0 = cols[idx]
                    nc.tensor.matmul(
                        s2[:, off + c0 : off + 512],
                        kT[p][64 * h : 64 * h + 64, P * j : P * (j + 1)],
                        q_ap[:, c0:512],
                        start=True,
                        stop=True,
                    )
                nc.scalar.activation(
                    p2[:, cols[0] : 1024], s2[:, cols[0] : 1024], EXPF
                )
                return p2, cols

            def av_pair(p, h, tau, a, o_ps, p2, cols):
                hl = 2 * p + h
                n_chunks = 4 * tau + 4
                for idx in range(2):
                    j = a + idx
                    t = j - 4 * tau
                    off = 512 * idx
                    c0 = cols[idx]
                    if t >= 0:
                        _, mk, mc = stair[t]
                        mt = dmask if mk == "d" else mask3
                        mw = P if mk == "d" else 256
                        nc.vector.tensor_mul(
                            p2[:, off + mc : off + mc + mw],
                            p2[:, off + mc : off + mc + mw],
                            mt[:, :mw],
                        )
                    nc.tensor.matmul(
                        o_ps[0:DV, c0:512],
                        vx[j][:, DV * hl : DV * (hl + 1)],
                        p2[:, off + c0 : off + 512],
                        start=(j == 0),
                        stop=(j == n_chunks - 1),
                    )

            def finish_group(p, h, tau, o_ps):
                qs = 512 * tau
                o_sb = osp.tile([DV, 512], f32, tag="os", name="o_sb")
                nc.scalar.activation(
                    o_sb[:], o_ps[0:DV, :], mybir.ActivationFunctionType.Copy
                )
                pending.append((o_sb, yT[p][64 * h : 64 * h + 64, qs : qs + 512]))

            def attn_group_pair(p, tau):
                n_chunks = 4 * tau + 4
                for h in range(2):
                    o_ps = pso.tile([P, 512], f32, tag="o", name="o_ps")
                    for a in range(0, n_chunks, 2):
                        p2, cols = s_pair(p, h, tau, a, o_ps)
                        av_pair(p, h, tau, a, o_ps, p2, cols)
                    finish_group(p, h, tau, o_ps)

            def project_rows(qt_lo, qt_hi, part, half, y_off):
                # projection for q-row tiles [qt_lo, qt_hi) + ReduceScatter
                for qt_i in range(qt_lo, qt_hi):
                    yp_sb = ypp.tile([P, E], f32, tag="yp")
                    for ec in range(2):
                        acc = psa.tile([P, 512], f32, tag="a", name="acc")
                        for p in range(NPAIR):
                            nc.tensor.matmul(
                                acc[:],
                                yT[p][:, P * qt_i : P * (qt_i + 1)],
                                wp_sb[:, p, 512 * ec : 512 * (ec + 1)],
                                start=(p == 0),
                                stop=(p == NPAIR - 1),
                            )
                        nc.vector.tensor_add(
                            yp_sb[:, 512 * ec : 512 * (ec + 1)],
                            acc[:],
                            bp_bc[:, 512 * ec : 512 * (ec + 1)],
                        )
                    nc.sync.dma_start(
                        part[P * (qt_i - qt_lo) : P * (qt_i - qt_lo + 1), :],
                        yp_sb[:],
                    )
                nc.gpsimd.collective_compute(
                    "ReduceScatter",
                    mybir.AluOpType.add,
                    replica_groups=[[0, 1], [2, 3], [4, 5], [6, 7]],
                    ins=[part.opt()],
                    outs=[half.opt()],
                )
                n_half = (qt_hi - qt_lo) * P // 2
                nc.gpsimd.dma_start(y_d[y_off : y_off + n_half, :], half[:])

            # prologue: QKV for the first q/k slice
            for u in kq_units(0) + v_units(0):
                u()
            # projection weights (needed from the first proj, ~100us in)
            wp_sb = wpp.tile([P, NPAIR, E], f32r)
            nc.sync.dma_start(wp_sb[:], wp_d.rearrange("(o p) m -> p o m", p=P))

            # filler schedule: QKV of slice ks+1 interleaves with tau=ks's
            # attention, three units ahead of each group-pair
            for tau in range(NKS):
                fillers = kq_units(tau + 1) + v_units(tau + 1) if tau < 3 else []
                for p in range(NPAIR):
                    for _ in range(3):
                        if fillers:
                            fillers.pop(0)()
                    attn_group_pair(p, tau)
                    # flush the PREVIOUS pair's normalize now: its
                    # reciprocal has had a whole pair's runtime to finish,
                    # so the broadcast matmuls don't stall the PE
                    if len(pending) >= 6:
                        flush_normalize()
                for u in fillers:
                    u()
                while pending:
                    flush_normalize()
                # ---- projection + ReduceScatter for this q-slice ----------
                if tau < 3:
                    project_rows(
                        4 * tau, 4 * tau + 4, y_parts[tau], y_halves[tau], 256 * tau
                    )
                else:
                    project_rows(12, 16, y_parts[3], y_halves[3], 768)

    nc.finalize()
    return nc


def _get_program():
    if "nc" not in _CACHED:
        _CACHED["nc"] = _build_program()
    return _CACHED["nc"]


def kernel(x, w_qkv, b_qkv, w_proj, b_proj, trace=False):
    x = np.asarray(x, dtype=np.float32)
    w_qkv = np.asarray(w_qkv, dtype=np.float32)
    b_qkv = np.asarray(b_qkv, dtype=np.float32)
    w_proj = np.asarray(w_proj, dtype=np.float32)
    b_proj = np.asarray(b_proj, dtype=np.float32)

    wq, wk, wv = w_qkv[:, :E], w_qkv[:, E : 2 * E], w_qkv[:, 2 * E :]
    bq, bk, bv = b_qkv[:E], b_qkv[E : 2 * E], b_qkv[2 * E :]
    scale = 1.0 / np.sqrt(np.float32(D))
    b16 = ml_dtypes.bfloat16

    in_maps = []
    for c in range(8):
        b, g = divmod(c, 2)
        sl = slice(g * HE, (g + 1) * HE)
        in_maps.append(
            {
                "xT": np.ascontiguousarray(x[b].T).astype(b16),
                "wq": np.ascontiguousarray(wq[:, sl] * scale).astype(b16),
                "wk": np.ascontiguousarray(wk[:, sl]).astype(b16),
                "wv": np.ascontiguousarray(wv[:, sl]).astype(b16),
                "wp": np.ascontiguousarray(w_proj[sl, :]),
                "bq": np.ascontiguousarray(bq[sl] * scale),
                "bk": np.ascontiguousarray(bk[sl]),
                "bv": np.ascontiguousarray(bv[sl][None, :]),
                "bp": np.ascontiguousarray((b_proj * 0.5)[None, :]),
            }
        )

    nc = _get_program()
    res = run_bass_kernel_spmd(nc, in_maps, list(range(8)), trace=trace)

    out = np.empty((B, S, E), dtype=np.float32)
    for c in range(8):
        b, g = divmod(c, 2)
        yo = res.results[c]["y_out"]
        # chunk j covers global rows 512j..512j+512; this core got the
        # g-th half of each chunk; the last chunk was split in two 256s
        for j in range(4):
            out[b, 512 * j + 256 * g : 512 * j + 256 * (g + 1), :] = yo[
                256 * j : 256 * (j + 1)
            ]
    if trace:
        return out, res
    return out


# revision 42
# speedup vs baseline: 1.0640x; 1.0640x over previous
"""Causal self-attention (B=4, S=2048, E=1024, H=16) on 8 TRN2 NeuronCores.

Sharding: data parallel on batch (4) x tensor parallel on heads (2 groups of 8).
Core c handles batch c//2, heads (c%2)*8..(c%2)*8+8. Each core computes its
heads' QKV projections, causal attention, and a partial output projection
(w_proj rows for its heads, with b_proj/2 folded in); core pairs then
ReduceScatter-add the partials so each core finishes half the rows of its
batch. No transposes on device: scores are computed as S^T = K @ Q^T, the
softmax denominator comes from a ones-column appended to V, and O^T stacked
over heads is exactly the lhsT the projection matmul needs.

QKV + attention run in bf16 (the PE's full-rate dtype for the K=64/M=65
attention shapes); the output projection accumulates y^T in float32r.
Work is interleaved: QKV per 512-column slice, then per 512-row q-slice
attention -> projection -> pairwise ReduceScatter, so the collectives and
the PE/ACT pipelines overlap.
"""

import sys

sys.path.insert(0, "/opt/trn_rl_repo")

import ml_dtypes
import numpy as np

import concourse.mybir as mybir
import concourse.tile as tile
from concourse import bacc
from concourse.bass_utils import run_bass_kernel_spmd

B, S, E, H, D = 4, 2048, 1024, 16, 64
P = 128
HPC = 8  # heads per core
NPAIR = HPC // 2  # head pairs per core (2 heads share a 128-partition tile)
HE = HPC * D  # 512: embedding slice owned by one core
NKT = S // P  # 16 k-row tiles
NKS = S // 512  # 4 sequence slices of 512
NEC = E // P  # 8 contraction chunks
DV = D + 1  # V columns per head incl. ones column
f32 = mybir.dt.float32
f32r = mybir.dt.float32r
bf16 = mybir.dt.bfloat16
EXPF = mybir.ActivationFunctionType.Exp

_CACHED = {}


def _build_program():
    nc = bacc.Bacc(None, target_bir_lowering=False)

    xT_d = nc.dram_tensor("xT", [E, S], bf16, kind="ExternalInput")
    wq_d = nc.dram_tensor("wq", [E, HE], bf16, kind="ExternalInput")
    wk_d = nc.dram_tensor("wk", [E, HE], bf16, kind="ExternalInput")
    wv_d = nc.dram_tensor("wv", [E, HE], bf16, kind="ExternalInput")
    wp_d = nc.dram_tensor("wp", [HE, E], f32r, kind="ExternalInput")
    bq_d = nc.dram_tensor("bq", [HE], f32, kind="ExternalInput")
    bk_d = nc.dram_tensor("bk", [HE], f32, kind="ExternalInput")
    bv_d = nc.dram_tensor("bv", [1, HE], f32r, kind="ExternalInput")
    bp_d = nc.dram_tensor("bp", [1, E], f32r, kind="ExternalInput")
    y_d = nc.dram_tensor("y_out", [S // 2, E], f32, kind="ExternalOutput")

    with tile.TileContext(nc) as tc:
        with (
            tc.tile_pool(name="const", bufs=1) as cst,
            tc.tile_pool(name="kt", bufs=1) as ktp,
            tc.tile_pool(name="qt", bufs=1) as qtp,
            tc.tile_pool(name="vx", bufs=1) as vxp,
            tc.tile_pool(name="yt", bufs=1) as ytp,
            tc.tile_pool(name="xs", bufs=2) as xsp,
            tc.tile_pool(name="wgt", bufs=1) as wgp,
            tc.tile_pool(name="wp", bufs=1) as wpp,
            tc.tile_pool(name="pt", bufs=5) as ptp,
            tc.tile_pool(name="sm", bufs=3) as smp,
            tc.tile_pool(name="os", bufs=6) as osp,
            tc.tile_pool(name="yp", bufs=3) as ypp,
            tc.tile_pool(name="ps_a", bufs=2, space="PSUM") as psa,
            tc.tile_pool(name="ps_s", bufs=2, space="PSUM") as pss,
            tc.tile_pool(name="ps_o", bufs=2, space="PSUM") as pso,
            tc.tile_pool(name="dram", bufs=1, space="DRAM") as dramp,
        ):
            # ---- constants -------------------------------------------------
            ones32 = cst.tile([P, P], f32)
            nc.gpsimd.memset(ones32[:], 1.0)
            ones_row = cst.tile([1, P], f32r)
            nc.vector.tensor_copy(ones_row[:], ones32[0:1, :])
            ones65 = cst.tile([65, 64], f32r)
            nc.vector.tensor_copy(ones65[:], ones32[0:65, 0:64])
            onescol = cst.tile([P, HPC], bf16)
            nc.vector.tensor_copy(onescol[:], ones32[:, :HPC])

            # diag mask [128,128]: keep (1.0) where col >= row
            dmask32 = cst.tile([P, P], f32)
            nc.gpsimd.memset(dmask32[:], 1.0)
            nc.gpsimd.affine_select(
                out=dmask32[:],
                in_=dmask32[:],
                compare_op=mybir.AluOpType.is_ge,
                fill=0.0,
                base=0,
                pattern=[[1, P]],
                channel_multiplier=-1,
            )
            dmask = cst.tile([P, P], bf16)
            nc.vector.tensor_copy(dmask[:], dmask32[:])
            # mask3 [128,256]: keep where col - 128 >= row (zeros | diag)
            m3_32 = cst.tile([P, 256], f32)
            nc.gpsimd.memset(m3_32[:], 1.0)
            nc.gpsimd.affine_select(
                out=m3_32[:],
                in_=m3_32[:],
                compare_op=mybir.AluOpType.is_ge,
                fill=0.0,
                base=-128,
                pattern=[[1, 256]],
                channel_multiplier=-1,
            )
            mask3 = cst.tile([P, 256], bf16)
            nc.vector.tensor_copy(mask3[:], m3_32[:])

            # persistent activations
            kT = [ktp.tile([P, S], bf16, tag=f"kt{p}", name=f"kt{p}") for p in range(NPAIR)]
            qT = [qtp.tile([P, S], bf16, tag=f"qt{p}", name=f"qt{p}") for p in range(NPAIR)]
            vx = [vxp.tile([P, HPC * DV], bf16, tag=f"vx{k}", name=f"vx{k}") for k in range(NKT)]
            yT = [ytp.tile([P, S], f32r, tag=f"yt{p}", name=f"yt{p}") for p in range(NPAIR)]

            # qkv weights (bf16, resident); xs slice 0 prefetched between
            # wk and wq so the first K matmuls start as early as possible
            wk_sb = wgp.tile([P, NEC, HE], bf16, tag="wk", name="wk_sb")
            wk_r = wk_d.rearrange("(o p) m -> p o m", p=P)
            nc.sync.dma_start(wk_sb[:, 0:2], wk_r[:, 0:2])
            xs0 = xsp.tile([P, NEC, 512], bf16, tag="xs", name="xs0")
            xs0_r = xT_d[:, 0:512].rearrange("(o p) s -> p o s", p=P)
            nc.sync.dma_start(xs0[:, 0:2], xs0_r[:, 0:2])
            nc.sync.dma_start(wk_sb[:, 2:NEC], wk_r[:, 2:NEC])
            nc.sync.dma_start(xs0[:, 2:NEC], xs0_r[:, 2:NEC])
            wq_sb = wgp.tile([P, NEC, HE], bf16, tag="wq", name="wq_sb")
            nc.sync.dma_start(wq_sb[:], wq_d.rearrange("(o p) m -> p o m", p=P))
            wv_sb = wgp.tile([P, NEC, HE], bf16, tag="wv", name="wv_sb")
            nc.sync.dma_start(wv_sb[:], wv_d.rearrange("(o p) m -> p o m", p=P))

            bq_sb = cst.tile([P, NPAIR], f32)
            nc.sync.dma_start(bq_sb[:], bq_d.rearrange("(o p) -> p o", p=P))
            bk_sb = cst.tile([P, NPAIR], f32)
            nc.sync.dma_start(bk_sb[:], bk_d.rearrange("(o p) -> p o", p=P))
            bv_row = cst.tile([1, HE], f32r)
            nc.sync.dma_start(bv_row[:], bv_d[:])
            bp_row = cst.tile([1, E], f32r)
            nc.sync.dma_start(bp_row[:], bp_d[:])

            # ---- emission-order software pipeline --------------------------
            # Tile executes ~in emission order per engine, so QKV slice ks+1
            # is emitted in unit-sized chunks between attention group-pairs
            # of q-slice tau=ks: the dense QKV matmuls fill the PE while the
            # ACT-bound attention pipeline runs, keeping the HAM clock warm.
            xs_tiles = {0: xs0}

            def xs_for(ks):
                if ks not in xs_tiles:
                    xs = xsp.tile([P, NEC, 512], bf16, tag="xs", name=f"xs{ks}")
                    nc.sync.dma_start(
                        xs[:],
                        xT_d[:, 512 * ks : 512 * (ks + 1)].rearrange(
                            "(o p) s -> p o s", p=P
                        ),
                    )
                    xs_tiles[ks] = xs
                return xs_tiles[ks]

            def kq_unit(ks, p, w_sb, b_sb, dst):
                xs = xs_for(ks)
                acc = psa.tile([P, 512], f32, tag="a", name="acc")
                for o in range(NEC):
                    nc.tensor.matmul(
                        acc[:],
                        w_sb[:, o, P * p : P * (p + 1)],
                        xs[:, o, :],
                        start=(o == 0),
                        stop=(o == NEC - 1),
                    )
                nc.vector.tensor_tensor(
                    dst[p][:, 512 * ks : 512 * (ks + 1)],
                    acc[:],
                    b_sb[:, p : p + 1].to_broadcast((P, 512)),
                    mybir.AluOpType.add,
                )

            def v_unit(ks, kl):
                xs = xs_for(ks)
                kt_i = 4 * ks + kl
                acc = psa.tile([P, HE], f32, tag="a", name="acc")
                for o in range(NEC):
                    nc.tensor.matmul(
                        acc[:],
                        xs[:, o, P * kl : P * (kl + 1)],
                        wv_sb[:, o, :],
                        start=(o == 0),
                        stop=(o == NEC - 1),
                    )
                vt = vx[kt_i].rearrange("p (h d) -> p h d", d=DV)
                nc.vector.tensor_tensor(
                    vt[:, :, 0:D],
                    acc[:].rearrange("p (h d) -> p h d", d=D),
                    bv_bc[:].rearrange("p (h d) -> p h d", d=D),
                    mybir.AluOpType.add,
                )
                nc.vector.tensor_copy(vt[:, :, D], onescol[:])

            def kq_units(ks):
                units = []
                for p in range(NPAIR):
                    units.append(lambda p=p: kq_unit(ks, p, wk_sb, bk_sb, kT))
                for p in range(NPAIR):
                    units.append(lambda p=p: kq_unit(ks, p, wq_sb, bq_sb, qT))
                return units

            def v_units(ks):
                return [lambda kl=kl: v_unit(ks, kl) for kl in range(4)]

            # staircase: (start col, mask tile, mask col) per sub-diag t
            stair = (
                (0, "d", 0),
                (128, "d", 128),
                (256, "d", 256),
                (256, "m3", 256),
            )
            # normalize in batches of 2: denominators gathered at partitions
            # 0 and 64 (quadrant-aligned for the broadcast matmul rhs), one
            # reciprocal per batch
            pending = []

            def flush_normalize():
                batch = pending[:2]
                del pending[:2]
                dn = smp.tile([65, 512], f32r, tag="dn", name="dn")
                for i, (o_sb, dst) in enumerate(batch):
                    nc.vector.tensor_copy(dn[64 * i : 64 * i + 1, :], o_sb[64:65, :])
                n = 64 * (len(batch) - 1) + 1
                with nc.allow_low_precision(reason="softmax recip"):
                    # one call over all gathered rows; the gap rows hold
                    # garbage and are reciprocal'd harmlessly
                    nc.vector.reciprocal(dn[0:n, :], dn[0:n, :])
                for i, (o_sb, dst) in enumerate(batch):
                    bc_ps = psa.tile([P, 512], f32, tag="a", name="bc_ps")
                    nc.tensor.matmul(
                        bc_ps[0:64, :],
                        ones65[64 * i : 64 * i + 1, :],
                        dn[64 * i : 64 * i + 1, :],
                        start=True,
                        stop=True,
                    )
                    bc_sb = smp.tile([64, 512], f32, tag="bcs")
                    nc.vector.tensor_copy(bc_sb[:], bc_ps[0:64, :])
                    nc.vector.tensor_mul(dst, o_sb[0:64, :], bc_sb[:])

            def s_pair(p, h, tau, a, o_ps):
                # S^T for chunks (a, a+1) -> one 2-bank psum tile, one Exp
                qs = 512 * tau
                q_ap = qT[p][64 * h : 64 * h + 64, qs : qs + 512]
                t0, t1 = a - 4 * tau, a + 1 - 4 * tau
                cols = (
                    0 if t0 < 0 else stair[t0][0],
                    0 if t1 < 0 else stair[t1][0],
                )
                s2 = pss.tile([P, 1024], f32, tag="s", name="s2")
                p2 = ptp.tile([P, 1024], bf16, tag="p", name="p2")
                for idx in range(2):
                    j = a + idx
                    off = 512 * idx
                    c0 = cols[idx]
                    nc.tensor.matmul(
                        s2[:, off + c0 : off + 512],
                        kT[p][64 * h : 64 * h + 64, P * j : P * (j + 1)],
                        q_ap[:, c0:512],
                        start=True,
                        stop=True,
                    )
                nc.scalar.activation(
                    p2[:, cols[0] : 1024], s2[:, cols[0] : 1024], EXPF
                )
                return p2, cols

            def av_pair(p, h, tau, a, o_ps, p2, cols):
                hl = 2 * p + h
                n_chunks = 4 * tau + 4
                for idx in range(2):
                    j = a + idx
                    t = j - 4 * tau
                    off = 512 * idx
                    c0 = cols[idx]
                    if t >= 0:
                        _, mk, mc = stair[t]
                        mt = dmask if mk == "d" else mask3
                        mw = P if mk == "d" else 256
                        nc.vector.tensor_mul(
                            p2[:, off + mc : off + mc + mw],
                            p2[:, off + mc : off + mc + mw],
                            mt[:, :mw],
                        )
                    nc.tensor.matmul(
                        o_ps[0:DV, c0:512],
                        vx[j][:, DV * hl : DV * (hl + 1)],
                        p2[:, off + c0 : off + 512],
                        start=(j == 0),
                        stop=(j == n_chunks - 1),
                    )

            def finish_group(p, h, tau, o_ps):
                qs = 512 * tau
                o_sb = osp.tile([DV, 512], f32, tag="os", name="o_sb")
                nc.scalar.activation(
                    o_sb[:], o_ps[0:DV, :], mybir.ActivationFunctionType.Copy
                )
                pending.append((o_sb, yT[p][64 * h : 64 * h + 64, qs : qs + 512]))

            def attn_group_pair(p, tau):
                n_chunks = 4 * tau + 4
                for h in range(2):
                    o_ps = pso.tile([P, 512], f32, tag="o", name="o_ps")
                    for a in range(0, n_chunks, 2):
                        p2, cols = s_pair(p, h, tau, a, o_ps)
                        av_pair(p, h, tau, a, o_ps, p2, cols)
                    finish_group(p, h, tau, o_ps)

            def project_rows(qt_lo, qt_hi, part, half, y_off):
                # projection for q-row tiles [qt_lo, qt_hi) + ReduceScatter
                for qt_i in range(qt_lo, qt_hi):
                    yp_sb = ypp.tile([P, E], f32, tag="yp")
                    for ec in range(2):
                        acc = psa.tile([P, 512], f32, tag="a", name="acc")
                        for p in range(NPAIR):
                            nc.tensor.matmul(
                                acc[:],
                                yT[p][:, P * qt_i : P * (qt_i + 1)],
                                wp_sb[:, p, 512 * ec : 512 * (ec + 1)],
                                start=(p == 0),
                                stop=(p == NPAIR - 1),
                            )
                        nc.vector.tensor_add(
                            yp_sb[:, 512 * ec : 512 * (ec + 1)],
                            acc[:],
                            bp_bc[:, 512 * ec : 512 * (ec + 1)],
                        )
                    nc.sync.dma_start(
                        part[P * (qt_i - qt_lo) : P * (qt_i - qt_lo + 1), :],
                        yp_sb[:],
                    )
                nc.gpsimd.collective_compute(
                    "ReduceScatter",
                    mybir.AluOpType.add,
                    replica_groups=[[0, 1], [2, 3], [4, 5], [6, 7]],
                    ins=[part.opt()],
                    outs=[half.opt()],
                )
                n_half = (qt_hi - qt_lo) * P // 2
                nc.gpsimd.dma_start(y_d[y_off : y_off + n_half, :], half[:])

            # prologue: K and Q for the first slice start the PE as soon
            # as the first weight chunks land; the bias broadcasts (whose
            # rows arrive behind the big weight DMAs) and V follow
            for u in kq_units(0):
                u()
            # ---- bias broadcast rows -> [128, *] tiles via K=1 matmul ------
            bv_bc = cst.tile([P, HE], f32)
            bp_bc = cst.tile([P, E], f32)
            bvp = psa.tile([P, HE], f32, tag="a")
            nc.tensor.matmul(bvp[:], ones_row[:], bv_row[:], start=True, stop=True)
            nc.vector.tensor_copy(bv_bc[:], bvp[:])
            for i in range(2):
                bpp = psa.tile([P, 512], f32, tag="a")
                nc.tensor.matmul(
                    bpp[:],
                    ones_row[:],
                    bp_row[:, 512 * i : 512 * (i + 1)],
                    start=True,
                    stop=True,
                )
                nc.vector.tensor_copy(bp_bc[:, 512 * i : 512 * (i + 1)], bpp[:])


            # ReduceScatter buffers: 3 full 512-row slices + the last
            # slice split 256/128/128 to shrink the serial tail
            _sizes = [512, 512, 512, 512]
            y_parts = [
                dramp.tile([n, E], f32, name=f"ypart{j}")
                for j, n in enumerate(_sizes)
            ]
            y_halves = [
                dramp.tile([n // 2, E], f32, name=f"yhalf{j}")
                for j, n in enumerate(_sizes)
            ]

            for u in v_units(0):
                u()
            # projection weights (needed from the first proj, ~100us in)
            wp_sb = wpp.tile([P, NPAIR, E], f32r)
            nc.sync.dma_start(wp_sb[:], wp_d.rearrange("(o p) m -> p o m", p=P))

            # filler schedule: QKV of slice ks+1 interleaves with tau=ks's
            # attention, three units ahead of each group-pair
            for tau in range(NKS):
                fillers = kq_units(tau + 1) + v_units(tau + 1) if tau < 3 else []
                for p in range(NPAIR):
                    take = 3 if p < NPAIR - 1 else 2
                    for _ in range(take):
                        if fillers:
                            fillers.pop(0)()
                    attn_group_pair(p, tau)
                    # flush the PREVIOUS pair's normalize now: its
                    # reciprocal has had a whole pair's runtime to finish,
                    # so the broadcast matmuls don't stall the PE
                    if len(pending) >= 6:
                        flush_normalize()
                # tau end: oldest batch's reciprocal is ready; the last
                # batch's reciprocal hides behind the reserved filler
                flush_normalize()
                for u in fillers:
                    u()
                while pending:
                    flush_normalize()
                # ---- projection + ReduceScatter for this q-slice ----------
                if tau < 3:
                    project_rows(
                        4 * tau, 4 * tau + 4, y_parts[tau], y_halves[tau], 256 * tau
                    )
                else:
                    project_rows(12, 16, y_parts[3], y_halves[3], 768)

    nc.finalize()
    return nc


def _get_program():
    if "nc" not in _CACHED:
        _CACHED["nc"] = _build_program()
    return _CACHED["nc"]


def kernel(x, w_qkv, b_qkv, w_proj, b_proj, trace=False):
    x = np.asarray(x, dtype=np.float32)
    w_qkv = np.asarray(w_qkv, dtype=np.float32)
    b_qkv = np.asarray(b_qkv, dtype=np.float32)
    w_proj = np.asarray(w_proj, dtype=np.float32)
    b_proj = np.asarray(b_proj, dtype=np.float32)

    wq, wk, wv = w_qkv[:, :E], w_qkv[:, E : 2 * E], w_qkv[:, 2 * E :]
    bq, bk, bv = b_qkv[:E], b_qkv[E : 2 * E], b_qkv[2 * E :]
    scale = 1.0 / np.sqrt(np.float32(D))
    b16 = ml_dtypes.bfloat16

    in_maps = []
    for c in range(8):
        b, g = divmod(c, 2)
        sl = slice(g * HE, (g + 1) * HE)
        in_maps.append(
            {
                "xT": np.ascontiguousarray(x[b].T).astype(b16),
                "wq": np.ascontiguousarray(wq[:, sl] * scale).astype(b16),
                "wk": np.ascontiguousarray(wk[:, sl]).astype(b16),
                "wv": np.ascontiguousarray(wv[:, sl]).astype(b16),
                "wp": np.ascontiguousarray(w_proj[sl, :]),
                "bq": np.ascontiguousarray(bq[sl] * scale),
                "bk": np.ascontiguousarray(bk[sl]),
                "bv": np.ascontiguousarray(bv[sl][None, :]),
                "bp": np.ascontiguousarray((b_proj * 0.5)[None, :]),
            }
        )

    nc = _get_program()
    res = run_bass_kernel_spmd(nc, in_maps, list(range(8)), trace=trace)

    out = np.empty((B, S, E), dtype=np.float32)
    for c in range(8):
        b, g = divmod(c, 2)
        yo = res.results[c]["y_out"]
        # chunk j covers global rows 512j..512j+512; this core got the
        # g-th half of each chunk; the last chunk was split in two 256s
        for j in range(4):
            out[b, 512 * j + 256 * g : 512 * j + 256 * (g + 1), :] = yo[
                256 * j : 256 * (j + 1)
            ]
    if trace:
        return out, res
    return out


# revision 43
# speedup vs baseline: 1.0736x; 1.0090x over previous
"""Causal self-attention (B=4, S=2048, E=1024, H=16) on 8 TRN2 NeuronCores.

Sharding: data parallel on batch (4) x tensor parallel on heads (2 groups of 8).
Core c handles batch c//2, heads (c%2)*8..(c%2)*8+8. Each core computes its
heads' QKV projections, causal attention, and a partial output projection
(w_proj rows for its heads, with b_proj/2 folded in); core pairs then
ReduceScatter-add the partials so each core finishes half the rows of its
batch. No transposes on device: scores are computed as S^T = K @ Q^T, the
softmax denominator comes from a ones-column appended to V, and O^T stacked
over heads is exactly the lhsT the projection matmul needs.

QKV + attention run in bf16 (the PE's full-rate dtype for the K=64/M=65
attention shapes); the output projection accumulates y^T in float32r.
Work is interleaved: QKV per 512-column slice, then per 512-row q-slice
attention -> projection -> pairwise ReduceScatter, so the collectives and
the PE/ACT pipelines overlap.
"""

import sys

sys.path.insert(0, "/opt/trn_rl_repo")

import ml_dtypes
import numpy as np

import concourse.mybir as mybir
import concourse.tile as tile
from concourse import bacc
from concourse.bass_utils import run_bass_kernel_spmd

B, S, E, H, D = 4, 2048, 1024, 16, 64
P = 128
HPC = 8  # heads per core
NPAIR = HPC // 2  # head pairs per core (2 heads share a 128-partition tile)
HE = HPC * D  # 512: embedding slice owned by one core
NKT = S // P  # 16 k-row tiles
NKS = S // 512  # 4 sequence slices of 512
NEC = E // P  # 8 contraction chunks
DV = D + 1  # V columns per head incl. ones column
f32 = mybir.dt.float32
f32r = mybir.dt.float32r
bf16 = mybir.dt.bfloat16
EXPF = mybir.ActivationFunctionType.Exp

_CACHED = {}


def _build_program():
    nc = bacc.Bacc(None, target_bir_lowering=False)

    xT_d = nc.dram_tensor("xT", [E, S], bf16, kind="ExternalInput")
    wq_d = nc.dram_tensor("wq", [E, HE], bf16, kind="ExternalInput")
    wk_d = nc.dram_tensor("wk", [E, HE], bf16, kind="ExternalInput")
    wv_d = nc.dram_tensor("wv", [E, HE], bf16, kind="ExternalInput")
    wp_d = nc.dram_tensor("wp", [HE, E], f32r, kind="ExternalInput")
    bq_d = nc.dram_tensor("bq", [HE], f32, kind="ExternalInput")
    bk_d = nc.dram_tensor("bk", [HE], f32, kind="ExternalInput")
    bv_d = nc.dram_tensor("bv", [1, HE], f32r, kind="ExternalInput")
    bp_d = nc.dram_tensor("bp", [1, E], f32r, kind="ExternalInput")
    y_d = nc.dram_tensor("y_out", [S // 2, E], f32, kind="ExternalOutput")

    with tile.TileContext(nc) as tc:
        with (
            tc.tile_pool(name="const", bufs=1) as cst,
            tc.tile_pool(name="kt", bufs=1) as ktp,
            tc.tile_pool(name="qt", bufs=1) as qtp,
            tc.tile_pool(name="vx", bufs=1) as vxp,
            tc.tile_pool(name="yt", bufs=1) as ytp,
            tc.tile_pool(name="xs", bufs=2) as xsp,
            tc.tile_pool(name="wgt", bufs=1) as wgp,
            tc.tile_pool(name="wp", bufs=1) as wpp,
            tc.tile_pool(name="pt", bufs=6) as ptp,
            tc.tile_pool(name="sm", bufs=4) as smp,
            tc.tile_pool(name="os", bufs=6) as osp,
            tc.tile_pool(name="yp", bufs=2) as ypp,
            tc.tile_pool(name="ps_a", bufs=2, space="PSUM") as psa,
            tc.tile_pool(name="ps_s", bufs=2, space="PSUM") as pss,
            tc.tile_pool(name="ps_o", bufs=2, space="PSUM") as pso,
            tc.tile_pool(name="dram", bufs=1, space="DRAM") as dramp,
        ):
            # ---- constants -------------------------------------------------
            ones32 = cst.tile([P, P], f32)
            nc.gpsimd.memset(ones32[:], 1.0)
            ones_row = cst.tile([1, P], f32r)
            nc.vector.tensor_copy(ones_row[:], ones32[0:1, :])
            ones65 = cst.tile([65, 64], f32r)
            nc.vector.tensor_copy(ones65[:], ones32[0:65, 0:64])
            onescol = cst.tile([P, HPC], bf16)
            nc.vector.tensor_copy(onescol[:], ones32[:, :HPC])

            # diag mask [128,128]: keep (1.0) where col >= row
            dmask32 = cst.tile([P, P], f32)
            nc.gpsimd.memset(dmask32[:], 1.0)
            nc.gpsimd.affine_select(
                out=dmask32[:],
                in_=dmask32[:],
                compare_op=mybir.AluOpType.is_ge,
                fill=0.0,
                base=0,
                pattern=[[1, P]],
                channel_multiplier=-1,
            )
            dmask = cst.tile([P, P], bf16)
            nc.vector.tensor_copy(dmask[:], dmask32[:])
            # mask3 [128,256]: keep where col - 128 >= row (zeros | diag)
            m3_32 = cst.tile([P, 256], f32)
            nc.gpsimd.memset(m3_32[:], 1.0)
            nc.gpsimd.affine_select(
                out=m3_32[:],
                in_=m3_32[:],
                compare_op=mybir.AluOpType.is_ge,
                fill=0.0,
                base=-128,
                pattern=[[1, 256]],
                channel_multiplier=-1,
            )
            mask3 = cst.tile([P, 256], bf16)
            nc.vector.tensor_copy(mask3[:], m3_32[:])

            # persistent activations
            kT = [ktp.tile([P, S], bf16, tag=f"kt{p}", name=f"kt{p}") for p in range(NPAIR)]
            qT = [qtp.tile([P, S], bf16, tag=f"qt{p}", name=f"qt{p}") for p in range(NPAIR)]
            vx = [vxp.tile([P, HPC * DV], bf16, tag=f"vx{k}", name=f"vx{k}") for k in range(NKT)]
            yT = [ytp.tile([P, S], f32r, tag=f"yt{p}", name=f"yt{p}") for p in range(NPAIR)]

            # qkv weights (bf16, resident); xs slice 0 prefetched between
            # wk and wq so the first K matmuls start as early as possible
            wk_sb = wgp.tile([P, NEC, HE], bf16, tag="wk", name="wk_sb")
            wk_r = wk_d.rearrange("(o p) m -> p o m", p=P)
            nc.sync.dma_start(wk_sb[:, 0:2], wk_r[:, 0:2])
            xs0 = xsp.tile([P, NEC, 512], bf16, tag="xs", name="xs0")
            xs0_r = xT_d[:, 0:512].rearrange("(o p) s -> p o s", p=P)
            nc.sync.dma_start(xs0[:, 0:2], xs0_r[:, 0:2])
            nc.sync.dma_start(wk_sb[:, 2:NEC], wk_r[:, 2:NEC])
            nc.sync.dma_start(xs0[:, 2:NEC], xs0_r[:, 2:NEC])
            wq_sb = wgp.tile([P, NEC, HE], bf16, tag="wq", name="wq_sb")
            nc.sync.dma_start(wq_sb[:], wq_d.rearrange("(o p) m -> p o m", p=P))
            wv_sb = wgp.tile([P, NEC, HE], bf16, tag="wv", name="wv_sb")
            nc.sync.dma_start(wv_sb[:], wv_d.rearrange("(o p) m -> p o m", p=P))

            bq_sb = cst.tile([P, NPAIR], f32)
            nc.sync.dma_start(bq_sb[:], bq_d.rearrange("(o p) -> p o", p=P))
            bk_sb = cst.tile([P, NPAIR], f32)
            nc.sync.dma_start(bk_sb[:], bk_d.rearrange("(o p) -> p o", p=P))
            bv_row = cst.tile([1, HE], f32r)
            nc.sync.dma_start(bv_row[:], bv_d[:])
            bp_row = cst.tile([1, E], f32r)
            nc.sync.dma_start(bp_row[:], bp_d[:])

            # ---- emission-order software pipeline --------------------------
            # Tile executes ~in emission order per engine, so QKV slice ks+1
            # is emitted in unit-sized chunks between attention group-pairs
            # of q-slice tau=ks: the dense QKV matmuls fill the PE while the
            # ACT-bound attention pipeline runs, keeping the HAM clock warm.
            xs_tiles = {0: xs0}

            def xs_for(ks):
                if ks not in xs_tiles:
                    xs = xsp.tile([P, NEC, 512], bf16, tag="xs", name=f"xs{ks}")
                    nc.sync.dma_start(
                        xs[:],
                        xT_d[:, 512 * ks : 512 * (ks + 1)].rearrange(
                            "(o p) s -> p o s", p=P
                        ),
                    )
                    xs_tiles[ks] = xs
                return xs_tiles[ks]

            def kq_unit(ks, p, w_sb, b_sb, dst):
                xs = xs_for(ks)
                acc = psa.tile([P, 512], f32, tag="a", name="acc")
                for o in range(NEC):
                    nc.tensor.matmul(
                        acc[:],
                        w_sb[:, o, P * p : P * (p + 1)],
                        xs[:, o, :],
                        start=(o == 0),
                        stop=(o == NEC - 1),
                    )
                nc.vector.tensor_tensor(
                    dst[p][:, 512 * ks : 512 * (ks + 1)],
                    acc[:],
                    b_sb[:, p : p + 1].to_broadcast((P, 512)),
                    mybir.AluOpType.add,
                )

            def v_unit(ks, kl):
                xs = xs_for(ks)
                kt_i = 4 * ks + kl
                acc = psa.tile([P, HE], f32, tag="a", name="acc")
                for o in range(NEC):
                    nc.tensor.matmul(
                        acc[:],
                        xs[:, o, P * kl : P * (kl + 1)],
                        wv_sb[:, o, :],
                        start=(o == 0),
                        stop=(o == NEC - 1),
                    )
                vt = vx[kt_i].rearrange("p (h d) -> p h d", d=DV)
                nc.vector.tensor_tensor(
                    vt[:, :, 0:D],
                    acc[:].rearrange("p (h d) -> p h d", d=D),
                    bv_bc[:].rearrange("p (h d) -> p h d", d=D),
                    mybir.AluOpType.add,
                )
                nc.vector.tensor_copy(vt[:, :, D], onescol[:])

            def kq_units(ks):
                units = []
                for p in range(NPAIR):
                    units.append(lambda p=p: kq_unit(ks, p, wk_sb, bk_sb, kT))
                for p in range(NPAIR):
                    units.append(lambda p=p: kq_unit(ks, p, wq_sb, bq_sb, qT))
                return units

            def v_units(ks):
                return [lambda kl=kl: v_unit(ks, kl) for kl in range(4)]

            # staircase: (start col, mask tile, mask col) per sub-diag t
            stair = (
                (0, "d", 0),
                (128, "d", 128),
                (256, "d", 256),
                (256, "m3", 256),
            )
            # normalize in batches of 2: denominators gathered at partitions
            # 0 and 64 (quadrant-aligned for the broadcast matmul rhs), one
            # reciprocal per batch
            pending = []

            def flush_normalize():
                batch = pending[:2]
                del pending[:2]
                dn = smp.tile([65, 512], f32r, tag="dn", name="dn")
                for i, (o_sb, dst) in enumerate(batch):
                    nc.vector.tensor_copy(dn[64 * i : 64 * i + 1, :], o_sb[64:65, :])
                n = 64 * (len(batch) - 1) + 1
                with nc.allow_low_precision(reason="softmax recip"):
                    # one call over all gathered rows; the gap rows hold
                    # garbage and are reciprocal'd harmlessly
                    nc.vector.reciprocal(dn[0:n, :], dn[0:n, :])
                for i, (o_sb, dst) in enumerate(batch):
                    bc_ps = psa.tile([P, 512], f32, tag="a", name="bc_ps")
                    nc.tensor.matmul(
                        bc_ps[0:64, :],
                        ones65[64 * i : 64 * i + 1, :],
                        dn[64 * i : 64 * i + 1, :],
                        start=True,
                        stop=True,
                    )
                    bc_sb = smp.tile([64, 512], f32, tag="bcs")
                    nc.vector.tensor_copy(bc_sb[:], bc_ps[0:64, :])
                    nc.vector.tensor_mul(dst, o_sb[0:64, :], bc_sb[:])

            def s_pair(p, h, tau, a, o_ps):
                # S^T for chunks (a, a+1) -> one 2-bank psum tile, one Exp
                qs = 512 * tau
                q_ap = qT[p][64 * h : 64 * h + 64, qs : qs + 512]
                t0, t1 = a - 4 * tau, a + 1 - 4 * tau
                cols = (
                    0 if t0 < 0 else stair[t0][0],
                    0 if t1 < 0 else stair[t1][0],
                )
                s2 = pss.tile([P, 1024], f32, tag="s", name="s2")
                p2 = ptp.tile([P, 1024], bf16, tag="p", name="p2")
                for idx in range(2):
                    j = a + idx
                    off = 512 * idx
                    c0 = cols[idx]
                    nc.tensor.matmul(
                        s2[:, off + c0 : off + 512],
                        kT[p][64 * h : 64 * h + 64, P * j : P * (j + 1)],
                        q_ap[:, c0:512],
                        start=True,
                        stop=True,
                    )
                nc.scalar.activation(
                    p2[:, cols[0] : 1024], s2[:, cols[0] : 1024], EXPF
                )
                return p2, cols

            def av_pair(p, h, tau, a, o_ps, p2, cols):
                hl = 2 * p + h
                n_chunks = 4 * tau + 4
                for idx in range(2):
                    j = a + idx
                    t = j - 4 * tau
                    off = 512 * idx
                    c0 = cols[idx]
                    if t >= 0:
                        _, mk, mc = stair[t]
                        mt = dmask if mk == "d" else mask3
                        mw = P if mk == "d" else 256
                        nc.vector.tensor_mul(
                            p2[:, off + mc : off + mc + mw],
                            p2[:, off + mc : off + mc + mw],
                            mt[:, :mw],
                        )
                    nc.tensor.matmul(
                        o_ps[0:DV, c0:512],
                        vx[j][:, DV * hl : DV * (hl + 1)],
                        p2[:, off + c0 : off + 512],
                        start=(j == 0),
                        stop=(j == n_chunks - 1),
                    )

            def finish_group(p, h, tau, o_ps):
                qs = 512 * tau
                o_sb = osp.tile([DV, 512], f32, tag="os", name="o_sb")
                nc.scalar.activation(
                    o_sb[:], o_ps[0:DV, :], mybir.ActivationFunctionType.Copy
                )
                pending.append((o_sb, yT[p][64 * h : 64 * h + 64, qs : qs + 512]))

            def attn_group_pair(p, tau):
                n_chunks = 4 * tau + 4
                for h in range(2):
                    o_ps = pso.tile([P, 512], f32, tag="o", name="o_ps")
                    for a in range(0, n_chunks, 2):
                        p2, cols = s_pair(p, h, tau, a, o_ps)
                        av_pair(p, h, tau, a, o_ps, p2, cols)
                    finish_group(p, h, tau, o_ps)

            def project_rows(qt_lo, qt_hi, part, half, y_off):
                # projection for q-row tiles [qt_lo, qt_hi) + ReduceScatter
                for qt_i in range(qt_lo, qt_hi):
                    yp_sb = ypp.tile([P, E], f32, tag="yp")
                    for ec in range(2):
                        acc = psa.tile([P, 512], f32, tag="a", name="acc")
                        for p in range(NPAIR):
                            nc.tensor.matmul(
                                acc[:],
                                yT[p][:, P * qt_i : P * (qt_i + 1)],
                                wp_sb[:, p, 512 * ec : 512 * (ec + 1)],
                                start=(p == 0),
                                stop=(p == NPAIR - 1),
                            )
                        nc.vector.tensor_add(
                            yp_sb[:, 512 * ec : 512 * (ec + 1)],
                            acc[:],
                            bp_bc[:, 512 * ec : 512 * (ec + 1)],
                        )
                    nc.sync.dma_start(
                        part[P * (qt_i - qt_lo) : P * (qt_i - qt_lo + 1), :],
                        yp_sb[:],
                    )
                nc.gpsimd.collective_compute(
                    "ReduceScatter",
                    mybir.AluOpType.add,
                    replica_groups=[[0, 1], [2, 3], [4, 5], [6, 7]],
                    ins=[part.opt()],
                    outs=[half.opt()],
                )
                n_half = (qt_hi - qt_lo) * P // 2
                nc.gpsimd.dma_start(y_d[y_off : y_off + n_half, :], half[:])

            # prologue: K and Q for the first slice start the PE as soon
            # as the first weight chunks land; the bias broadcasts (whose
            # rows arrive behind the big weight DMAs) and V follow
            for u in kq_units(0):
                u()
            # ---- bias broadcast rows -> [128, *] tiles via K=1 matmul ------
            bv_bc = cst.tile([P, HE], f32)
            bp_bc = cst.tile([P, E], f32)
            bvp = psa.tile([P, HE], f32, tag="a")
            nc.tensor.matmul(bvp[:], ones_row[:], bv_row[:], start=True, stop=True)
            nc.vector.tensor_copy(bv_bc[:], bvp[:])
            for i in range(2):
                bpp = psa.tile([P, 512], f32, tag="a")
                nc.tensor.matmul(
                    bpp[:],
                    ones_row[:],
                    bp_row[:, 512 * i : 512 * (i + 1)],
                    start=True,
                    stop=True,
                )
                nc.vector.tensor_copy(bp_bc[:, 512 * i : 512 * (i + 1)], bpp[:])


            # ReduceScatter buffers: 3 full 512-row slices + the last
            # slice split 256/128/128 to shrink the serial tail
            _sizes = [512, 512, 512, 512]
            y_parts = [
                dramp.tile([n, E], f32, name=f"ypart{j}")
                for j, n in enumerate(_sizes)
            ]
            y_halves = [
                dramp.tile([n // 2, E], f32, name=f"yhalf{j}")
                for j, n in enumerate(_sizes)
            ]

            for u in v_units(0):
                u()
            # projection weights (needed from the first proj, ~100us in)
            wp_sb = wpp.tile([P, NPAIR, E], f32r)
            nc.sync.dma_start(wp_sb[:], wp_d.rearrange("(o p) m -> p o m", p=P))

            # filler schedule: QKV of slice ks+1 interleaves with tau=ks's
            # attention, three units ahead of each group-pair
            v3 = v_units(3)
            for tau in range(NKS):
                if tau < 2:
                    fillers = kq_units(tau + 1) + v_units(tau + 1)
                elif tau == 2:
                    fillers = kq_units(3) + v3[:2]
                else:
                    # the saved V(3) units are tau=3's only dense PE filler;
                    # their outputs feed every tau=3 staircase, so they land
                    # ahead of the first group
                    fillers = v3[2:]
                for p in range(NPAIR):
                    take = 3 if p < NPAIR - 1 else 2
                    for _ in range(take):
                        if fillers:
                            fillers.pop(0)()
                    attn_group_pair(p, tau)
                    # flush the PREVIOUS pair's normalize now: its
                    # reciprocal has had a whole pair's runtime to finish,
                    # so the broadcast matmuls don't stall the PE
                    if len(pending) >= 6:
                        flush_normalize()
                # tau end: oldest batch's reciprocal is ready; the last
                # batch's reciprocal hides behind the reserved filler
                flush_normalize()
                for u in fillers:
                    u()
                while pending:
                    flush_normalize()
                # ---- projection + ReduceScatter for this q-slice ----------
                if tau < 3:
                    project_rows(
                        4 * tau, 4 * tau + 4, y_parts[tau], y_halves[tau], 256 * tau
                    )
                else:
                    project_rows(12, 16, y_parts[3], y_halves[3], 768)

    nc.finalize()
    return nc


def _get_program():
    if "nc" not in _CACHED:
        _CACHED["nc"] = _build_program()
    return _CACHED["nc"]


def kernel(x, w_qkv, b_qkv, w_proj, b_proj, trace=False):
    x = np.asarray(x, dtype=np.float32)
    w_qkv = np.asarray(w_qkv, dtype=np.float32)
    b_qkv = np.asarray(b_qkv, dtype=np.float32)
    w_proj = np.asarray(w_proj, dtype=np.float32)
    b_proj = np.asarray(b_proj, dtype=np.float32)

    wq, wk, wv = w_qkv[:, :E], w_qkv[:, E : 2 * E], w_qkv[:, 2 * E :]
    bq, bk, bv = b_qkv[:E], b_qkv[E : 2 * E], b_qkv[2 * E :]
    scale = 1.0 / np.sqrt(np.float32(D))
    b16 = ml_dtypes.bfloat16

    in_maps = []
    for c in range(8):
        b, g = divmod(c, 2)
        sl = slice(g * HE, (g + 1) * HE)
        in_maps.append(
            {
                "xT": np.ascontiguousarray(x[b].T).astype(b16),
                "wq": np.ascontiguousarray(wq[:, sl] * scale).astype(b16),
                "wk": np.ascontiguousarray(wk[:, sl]).astype(b16),
                "wv": np.ascontiguousarray(wv[:, sl]).astype(b16),
                "wp": np.ascontiguousarray(w_proj[sl, :]),
                "bq": np.ascontiguousarray(bq[sl] * scale),
                "bk": np.ascontiguousarray(bk[sl]),
                "bv": np.ascontiguousarray(bv[sl][None, :]),
                "bp": np.ascontiguousarray((b_proj * 0.5)[None, :]),
            }
        )

    nc = _get_program()
    res = run_bass_kernel_spmd(nc, in_maps, list(range(8)), trace=trace)

    out = np.empty((B, S, E), dtype=np.float32)
    for c in range(8):
        b, g = divmod(c, 2)
        yo = res.results[c]["y_out"]
        # chunk j covers global rows 512j..512j+512; this core got the
        # g-th half of each chunk; the last chunk was split in two 256s
        for j in range(4):
            out[b, 512 * j + 256 * g : 512 * j + 256 * (g + 1), :] = yo[
                256 * j : 256 * (j + 1)
            ]
    if trace:
        return out, res
    return out
